# revision 23
# baseline (speedup 1.0000x reference)
"""GATv2 attention-pool kernel for 8 Trainium2 NeuronCores.

Algorithm
---------
Reference computes, per edge e with target node t(e):
    feats = q + k                                   [E, 64]
    logits[e,h] = sum_c feats[e,h*8+c] * A[c,h]     [E, 8]
    attn = segment_softmax(logits, targets)         [E, 8]
    out[n] = relu(segment_sum(q * attn))            [N, 64]

Logits are O(10) so exp() never overflows fp32/bf16; the segment-max shift
is unnecessary and softmax folds into two segment-SUMS sharing one pass:
    denom[n,h]  = sum_{e->n} exp(logits[e,h])
    pooled[n,:] = sum_{e->n} q[e,:] * exp(logits[e,h])
    out[n]      = relu(pooled[n]) / denom[n]        (relu commutes: denom>0)

Distribution: edges partitioned by target node (host-side sort), 100000
nodes split into 8 contiguous shards -> all segment reductions core-local,
no collectives.  Each shard's nodes are LPT-packed into windows of <= 32
nodes and <= 512 edges (4 subtiles of 128, ~0.6% slot padding); per
subtile the PE accumulates
    psum[32, 72] += S^T @ [q*ex | ex]
over the window's subtiles (S = host-built one-hot selector, streamed like
the data), then relu/divide once per node.

Key performance choices (vs a naive port):
- fp16 staging of q/k and bf16 ex/matmul operands halve HBM traffic and
  double DVE throughput (2x_1p mode needs 2-byte packed operands);
  rel-err vs the fp32 reference is ~8e-3 (tolerance 2e-2).
- fp8e4 one-hot selector: 0/1 is exact in fp8 and the PE accepts an fp8
  stationary operand against a bf16 moving one (HW-probed, err 2e-3);
  halves selector HBM traffic.
- HYBRID E/F split (the big one): the system is simultaneously DVE- and
  DMA-limited.  The last NF of each group's 56 subtiles ("F-subtiles")
  compute logits on the PE instead of DVE: a feat-major copy of q,k
  (features on partitions) is matmul'd against a zero-padded [128,32]
  weight whose band-a matmul lands at PSUM partitions [32a,32a+32)
  via tile_position, so after ACT exp (PSUM->SBUF) a DVE 32x32 block
  stream-transpose yields edge-major ex directly.  This removes
  fadd/wmul/tree (~94ns/subtile of DVE) at +16KB/subtile of DMA
  (feat-major qk + an edge-major q copy for qex).  NF balances the two
  rooflines.  The F chain runs TWO group-iterations ahead (matmuls at
  load time, exp+transpose emitted at iteration end) so the in-order
  DVE queue never waits on the PE->ACT->DVE chain.
- DMA split across BOTH HWDGE rings (SP + ACT) plus SWDGE (gpsimd) for
  the out-writes: a single ring head-of-line blocks loads behind the
  out-DMA's wait on the epilogue, capping sustained DMA at ~260GB/s;
  split rings reach ~290-350GB/s.
- h-major feature columns (c outer, h inner) make the per-head exp()
  broadcast and the 1/denom broadcast middle-axis: every DVE operand
  keeps a packed innermost dim and no broadcast is ever materialized.
- the c-reduction for E-subtile logits is a 3-step contiguous-halves
  add tree (tensor_reduce has no fast DVE mode; the tree runs at 2x).
- elementwise work runs on DVE; ACT does exp/relu, PE the matmuls.

Host work is index metadata + data layout only (argsort/packing of
targets, one-hot selector layout, gather + dtype cast of q/k rows into
slot order); all floating-point arithmetic runs on device.

Measured dead ends, do not retry blindly: GpSimd tensor-op offload
(qex share on Pool: TT runs 3.9ns/el AND concurrent Pool SBUF traffic
poisons DVE ops 2000->4200ns; 262->356us), per-subtile PE transposes
(LDWEIGHTS-bound), tensor_tensor divide (codegen reject), ACT Ln/Exp
reciprocal (crashes the core), fp8 staging of q/k (precision).
DVE per-op cost model that matches HW within 5%: (58 + FD/2)/0.96GHz
for 16-bit tensor_tensor (2x_1P cap), 1x for stream transpose/scan.
"""

import os
import sys

import numpy as np

N_NODES = 100000
N_EDGES = 1600000
H = 8
C = 8
HC = H * C
N_CORES = 8
SUB = 128
WIN = 32          # nodes per window
TW = 4            # subtiles per window
CAP = TW * SUB    # max edges per window
G = 14            # windows per device group (2 psum tiles)
PWIN = 7          # windows per psum tile ((7*72)*4B < 2KB bank)

FADD_DVE_SUBTILES = 999  # f-add subtiles on DVE (rest on GpSimd).  All-DVE
                         # measured fastest: any GpSimd share re-couples the
                         # Vector pipeline to the slow Q7 engine.
QEX_POOL_NUM = 0         # qex subtiles (of 56/group) computed on GpSimd.
                         # MEASURED DEAD END at 30: Pool TT runs 3.9ns/el
                         # (not 2.2) AND concurrent Pool traffic poisons DVE
                         # via the shared SBUF port (fadd/wmul 2000->4200ns;
                         # total 262->356us).  Keep 0.
DMUL_ENGINE = "vector"   # engine for out = relu(pooled) * rc ("gpsimd"
                         # measured slower, same port-contention mechanism)

# Hybrid E/F split: the LAST `NF` subtiles (per 56-subtile group) are
# "F-subtiles": their logits run on the PE from a feat-major copy of q,k
# (one matmul per 32-edge band, placed at PSUM partition base 32a via
# tile_position), exp on ACT straight from PSUM, and a DVE 32x32
# stream-transpose flips ex back to edge-major.  This removes fadd/wmul/
# tree from DVE for those subtiles at the cost of loading q twice for
# them (feat-major inside qkF + edge-major qE).  DVE is the bottleneck
# (245us busy of 262); the extra DMA rides unused DMA-engine headroom.
NF = 32                  # F-subtiles per full 56-subtile group
PF = 16                  # F-subtiles per PSUM chunk (16*32*4B = 2048B/bank)


def _split(Tg):
    nf = (Tg * NF // (G * TW)) // PF * PF
    return Tg - nf, nf


def _ensure_imports():
    try:
        import concourse.bass  # noqa: F401
    except ImportError:
        for p in ("/opt/trn_rl_repo", "/root/.axon_site/_ro/trn_rl_repo"):
            if os.path.isdir(p) and p not in sys.path:
                sys.path.insert(0, p)


def preprocess(targets):
    """Sort edges by target; LPT-pack each core's nodes into windows.

    Windows hold <= WIN nodes and <= CAP edges.  LPT (assign nodes in
    descending degree order to the least-loaded feasible window) packs to
    ~0.6% slot padding.  Returns (perms [n_cores, n_slots] edge ids (-1 =
    padding), rels [n_cores, n_slots] int8 local node id (-1 = padding),
    node_order [n_cores, n_win*WIN] node id per output row (-1 = unused),
    n_win).
    """
    import heapq

    npc = N_NODES // N_CORES
    order = np.argsort(targets, kind="stable")
    tsorted = targets[order]
    node_start = np.searchsorted(tsorted, np.arange(N_NODES + 1))
    deg = np.diff(node_start)

    def lpt(nodes, n_win):
        degs = deg[nodes]
        heap = [(0, 0, i) for i in range(n_win)]
        heapq.heapify(heap)
        assign = [[] for _ in range(n_win)]
        for nd in np.argsort(-degs, kind="stable"):
            dd = int(degs[nd])
            popped = []
            placed = False
            while heap:
                load, cnt, i = heapq.heappop(heap)
                if load + dd <= CAP and cnt + 1 <= WIN:
                    heapq.heappush(heap, (load + dd, cnt + 1, i))
                    assign[i].append(int(nodes[nd]))
                    placed = True
                    break
                popped.append((load, cnt, i))
            for p in popped:
                heapq.heappush(heap, p)
            if not placed:
                return None
        return assign

    # minimal feasible n_win per core, then re-pack all cores at the max
    packs, n_wins = [], []
    for c in range(N_CORES):
        nodes = np.arange(c * npc, (c + 1) * npc)
        n_win = int(np.ceil(max(deg[nodes].sum() / CAP, len(nodes) / WIN)))
        while True:
            a = lpt(nodes, n_win)
            if a is not None:
                break
            n_win += 1
        n_wins.append(n_win)
    n_win = max(n_wins)
    for c in range(N_CORES):
        nodes = np.arange(c * npc, (c + 1) * npc)
        a = lpt(nodes, n_win)
        assert a is not None
        packs.append(a)

    n_slots = n_win * CAP
    perms = np.full((N_CORES, n_slots), -1, dtype=np.int64)
    rels = np.full((N_CORES, n_slots), -1, dtype=np.int8)
    node_order = np.full((N_CORES, n_win * WIN), -1, dtype=np.int64)
    for c in range(N_CORES):
        for w, cur in enumerate(packs[c]):
            sb = w * CAP
            pos = 0
            for j, node in enumerate(cur):
                e0, e1 = node_start[node], node_start[node + 1]
                cnt = e1 - e0
                perms[c, sb + pos:sb + pos + cnt] = order[e0:e1]
                rels[c, sb + pos:sb + pos + cnt] = j
                pos += cnt
                node_order[c, w * WIN + j] = node
    return perms, rels, node_order, n_win


def _groups(n_win):
    gs, w0 = [], 0
    while w0 < n_win:
        g = min(G, n_win - w0)
        gs.append((w0, g))
        w0 += g
    return gs


def build_nc(n_win):
    """Build the single SPMD Bass program for one core's shard."""
    _ensure_imports()
    import concourse.bacc as bacc
    import concourse.mybir as mybir
    import concourse.tile as tile

    f32 = mybir.dt.float32
    f16 = mybir.dt.float16
    bf16 = mybir.dt.bfloat16
    f8 = mybir.dt.float8e4

    groups = _groups(n_win)
    AF = mybir.ActivationFunctionType
    OP = mybir.AluOpType

    splits = [_split(TW * g) for _, g in groups]
    offE = np.concatenate([[0], np.cumsum([e for e, _ in splits])])
    offF = np.concatenate([[0], np.cumsum([f for _, f in splits])])
    totE, totF = int(offE[-1]), int(offF[-1])

    nc = bacc.Bacc("TRN2", num_devices=N_CORES)
    qkD = nc.declare_dram_parameter("qk", [SUB, totE * SUB], f16, False)
    qkFD = nc.declare_dram_parameter(
        "qkf", [SUB, max(totF, 1) * SUB], f16, False)
    qED = nc.declare_dram_parameter(
        "qe", [SUB, max(totF, 1) * HC], f16, False)
    w128D = nc.declare_dram_parameter("w128", [SUB, 32], f16, False)
    # fp8e4 selector: one-hot 0/1 is exact in fp8 and the PE accepts an
    # fp8 stationary operand against a bf16 moving one (probed on HW,
    # rel err 2e-3) -> halves the selector's HBM traffic.
    sD = nc.declare_dram_parameter("sel", [SUB, n_win * TW * WIN], f8,
                                   False)
    wD = nc.declare_dram_parameter("wrow", [SUB, G * TW * HC], f16, False)
    outD = nc.declare_dram_parameter(
        "out", [WIN, n_win * HC], bf16, isOutput=True)

    with tile.TileContext(nc) as tc:
        with (
            tc.tile_pool(name="const", bufs=1) as cpool,
            tc.tile_pool(name="qk", bufs=4) as qkpool,
            tc.tile_pool(name="mid", bufs=3) as midpool,
            tc.tile_pool(name="mm", bufs=3) as mmpool,
            tc.tile_pool(name="fin", bufs=3) as finpool,
            tc.tile_pool(name="psum", bufs=6, space="PSUM") as ppool,
            tc.tile_pool(name="psumL", bufs=2, space="PSUM") as plpool,
        ):
            w_t = cpool.tile([SUB, G * TW * HC], f16)
            nc.sync.dma_start(out=w_t[:], in_=wD[:])
            w128_t = cpool.tile([SUB, 32], f16)
            nc.sync.dma_start(out=w128_t[:], in_=w128D[:])

            def emit_load(grp):
                w0, g, gk = grp
                Tg = TW * g
                nE, nF = splits[gk]
                st = {"grp": grp, "Tg": Tg, "nE": nE, "nF": nF}
                # qkF/qE first: the F logits chain consumes qkF this same
                # iteration (2 periods before the group's DVE stage)
                # two HWDGE rings in parallel: qkF+sel on the ACT ring,
                # qkE+qE on the SP ring, out-writes on SWDGE (gpsimd) --
                # a single ring head-of-line blocks loads behind the
                # out-DMA's wait on the epilogue.  qkE first: the E chain
                # (fadd) is the rampup critical path.
                if nE > 0:
                    qk_t = qkpool.tile([SUB, nE * SUB], f16, tag=f"qk{nE}")
                    nc.sync.dma_start(
                        out=qk_t[:],
                        in_=qkD[:, offE[gk] * SUB:(offE[gk] + nE) * SUB])
                    st["qk3"] = qk_t[:].rearrange("p (t c) -> p t c", c=SUB)
                if nF > 0:
                    qkF_t = qkpool.tile([SUB, nF * SUB], f16, tag=f"qkF{nF}")
                    nc.scalar.dma_start(
                        out=qkF_t[:],
                        in_=qkFD[:, offF[gk] * SUB:(offF[gk] + nF) * SUB])
                    qE_t = qkpool.tile([SUB, nF * HC], f16, tag=f"qE{nF}")
                    nc.sync.dma_start(
                        out=qE_t[:],
                        in_=qED[:, offF[gk] * HC:(offF[gk] + nF) * HC])
                    st["qkF"] = qkF_t
                    st["qE3"] = qE_t[:].rearrange("p (t c) -> p t c", c=HC)
                s_t = qkpool.tile([SUB, Tg, WIN], f8, tag=f"S{g}")
                nc.scalar.dma_start(
                    out=s_t[:], in_=sD[:, w0 * TW * WIN:(w0 + g) * TW * WIN])
                st["S"] = s_t
                # F-subtile logits on PE, immediately at load time: per
                # chunk of PF subtiles, band a covers edges [32a, 32a+32) of
                # each subtile and lands at PSUM partitions [32a, 32a+32)
                # (tile_position) so a 32x32 stream-transpose yields
                # edge-major ex later.
                pls = []
                for ci in range(st["nF"] // PF):
                    pl = plpool.tile([SUB, PF * 32], f32, tag="PL")
                    for a in range(4):
                        c0 = (ci * 4 + a) * PF * 32
                        nc.tensor.matmul(
                            pl[a * 32:(a + 1) * 32, :],
                            lhsT=w128_t[:],
                            rhs=qkF_t[:, c0:c0 + PF * 32],
                            start=True, stop=True,
                            tile_position=(0, a * 32),
                        )
                    pls.append(pl)
                st["pls"] = pls
                return st

            def emit_fexp_tx(s):
                # tail of the F logits chain (still 2 periods ahead of the
                # group's DVE stage): exp from PSUM on ACT, then the 32x32
                # block stream-transpose to edge-major.  Emitted at the very
                # END of the iteration so nothing this period waits on it.
                txs = []
                for pl in s["pls"]:
                    px = mmpool.tile([SUB, PF * 32], bf16, tag="PX")
                    nc.scalar.activation(out=px[:], in_=pl[:], func=AF.Exp)
                    tx = mmpool.tile([SUB, PF * 32], bf16, tag="TX", bufs=6)
                    nc.vector.transpose(tx[:], px[:])
                    txs.append(tx)
                s["txs"] = txs
                s["pls"] = None

            def emit_fadd(s):
                nE = s["nE"]
                if nE == 0:
                    return
                f_t = midpool.tile([SUB, nE * HC], f16, tag=f"f{nE}", bufs=3)
                fv = f_t[:].rearrange("p (t c) -> p t c", c=HC)
                nc.vector.tensor_add(
                    fv[:], s["qk3"][:, :, 0:HC], s["qk3"][:, :, HC:2 * HC])
                s["f"] = f_t

            def emit_logits(s):
                # E-subtiles: features are h-major (c outer, h inner): the
                # c-reduction tree adds contiguous halves (2x DVE mode)
                nE, nF = s["nE"], s["nF"]
                if nE > 0:
                    wf_t = midpool.tile([SUB, nE * HC], f16, tag=f"wf{nE}")
                    nc.vector.tensor_mul(wf_t[:], s["f"][:], w_t[:, :nE * HC])
                    wfv = wf_t[:].rearrange("p (t c) -> p t c", c=HC)
                    t1 = midpool.tile([SUB, nE, 32], f16, tag=f"t1{nE}")
                    nc.vector.tensor_add(
                        t1[:], wfv[:, :, 0:32], wfv[:, :, 32:64])
                    t2 = midpool.tile([SUB, nE, 16], f16, tag=f"t2{nE}")
                    nc.vector.tensor_add(
                        t2[:], t1[:, :, 0:16], t1[:, :, 16:32])
                    lg = midpool.tile([SUB, nE * H], f16, tag=f"lg{nE}")
                    nc.vector.tensor_add(
                        lg[:].rearrange("p (t h) -> p t h", h=H),
                        t2[:, :, 0:8], t2[:, :, 8:16])
                    s["lg"] = lg

            def emit_exp(s):
                Tg, nE = s["Tg"], s["nE"]
                m_t = mmpool.tile([SUB, Tg, HC + H], bf16, tag=f"M{Tg}")
                if nE > 0:
                    lg3 = s["lg"][:].rearrange("p (t h) -> p t h", h=H)
                    nc.scalar.activation(
                        out=m_t[:, 0:nE, HC:HC + H], in_=lg3, func=AF.Exp)
                for ci, tx in enumerate(s["txs"]):
                    t0 = nE + ci * PF
                    nc.vector.tensor_copy(
                        m_t[:, t0:t0 + PF, HC:HC + H],
                        tx[:].rearrange("p (t c) -> p t c", c=32)[:, :, 0:H])
                s["m"] = m_t

            def emit_qex_mm(s):
                # h-major: the per-head ex broadcast is along the MIDDLE (c)
                # axis, so every operand's innermost dim stays packed (2x)
                # and no materialized broadcast is needed.
                Tg, nE, nF = s["Tg"], s["nE"], s["nF"]
                m_t = s["m"]
                exv = m_t[:, :, HC:HC + H]
                if nE > 0:
                    nc.vector.tensor_mul(
                        m_t[:, 0:nE, 0:HC].rearrange(
                            "p t (cc h) -> p t cc h", h=H),
                        s["qk3"][:, :, 0:HC].rearrange(
                            "p t (cc h) -> p t cc h", h=H),
                        exv[:, 0:nE, None, :].to_broadcast([SUB, nE, C, H]),
                    )
                if nF > 0:
                    nc.vector.tensor_mul(
                        m_t[:, nE:Tg, 0:HC].rearrange(
                            "p t (cc h) -> p t cc h", h=H),
                        s["qE3"][:].rearrange(
                            "p t (cc h) -> p t cc h", h=H),
                        exv[:, nE:Tg, None, :].to_broadcast([SUB, nF, C, H]),
                    )
                w0, g, _ = s["grp"]
                n_ps = (g + PWIN - 1) // PWIN
                ps = []
                for pi in range(n_ps):
                    nw = min(PWIN, g - pi * PWIN)
                    p_t = ppool.tile([WIN, PWIN * (HC + H)], f32, tag="ps")
                    for wi in range(nw):
                        sub0 = (pi * PWIN + wi) * TW
                        pcols = slice(wi * (HC + H), (wi + 1) * (HC + H))
                        for t in range(TW):
                            nc.tensor.matmul(
                                p_t[:, pcols],
                                lhsT=s["S"][:, sub0 + t, :],
                                rhs=m_t[:, sub0 + t, :],
                                start=(t == 0),
                                stop=(t == TW - 1),
                            )
                    ps.append((p_t, nw))
                s["ps"] = ps

            def emit_epilogue(s):
                w0, g, _ = s["grp"]
                po = finpool.tile([WIN, g, HC + H], bf16, tag=f"po{g}")
                off = 0
                for p_t, nw in s["ps"]:
                    nc.scalar.activation(
                        out=po[:, off:off + nw, :],
                        in_=p_t[:, :nw * (HC + H)].rearrange(
                            "p (w j) -> p w j", j=HC + H),
                        func=AF.Relu,
                    )
                    off += nw
                # 1/denom via the single-pass approx (~18 bits, plenty for
                # the bf16 result); it needs fp32 in/out, staged through the
                # idle ACT engine.  denom>0 for every real node (no
                # zero-degree nodes reach here; padded rows are host-masked).
                dn = finpool.tile([WIN, g, H], f32, tag=f"dn{g}")
                nc.scalar.activation(out=dn[:], in_=po[:, :, HC:HC + H],
                                     func=AF.Copy)
                rcf = finpool.tile([WIN, g, H], f32, tag=f"rcf{g}")
                nc.vector.reciprocal_approx_fast(out=rcf[:], in_=dn[:])
                rc = finpool.tile([WIN, g, H], bf16, tag=f"rc{g}")
                nc.scalar.activation(out=rc[:], in_=rcf[:], func=AF.Copy)
                s["po"], s["rc"] = po, rc

            def emit_epi_b(s):
                w0, g, _ = s["grp"]
                o_t = finpool.tile([WIN, g, HC], bf16, tag=f"o{g}")
                dmul_eng = nc.gpsimd if DMUL_ENGINE == "gpsimd" else nc.vector
                dmul_eng.tensor_mul(
                    o_t[:].rearrange("p w (cc h) -> p w cc h", h=H),
                    s["po"][:, :, 0:HC].rearrange(
                        "p w (cc h) -> p w cc h", h=H),
                    s["rc"][:, :, None, :].to_broadcast([WIN, g, C, H]),
                )
                nc.gpsimd.dma_start(
                    out=outD[:, w0 * HC:(w0 + g) * HC], in_=o_t[:])

            # 3-deep software pipeline; epilogue runs 2 iterations behind so
            # no engine's first op of an iteration waits on a fresh product.
            # Steady-state per-iteration engine FIFOs:
            #   ACT:  relu(i-2), exp/expb(i)
            #   DVE:  recip(i-2), wmul/tree(i+1), fadd-share(i+2), qex(i)
            #   Pool: dmul(i-2), fadd-share(i+2)
            #   PE:   mm(i);  DMA: out(i-2), load(i+3)
            n = len(groups)
            st = [None] * n

            def stage(gi, fn):
                if 0 <= gi < n:
                    fn(st[gi])

            for gi in range(min(2, n)):
                st[gi] = emit_load((*groups[gi], gi))
            stage(0, emit_fexp_tx)
            stage(1, emit_fexp_tx)
            stage(0, emit_fadd)
            stage(0, emit_logits)
            for gi in range(n):
                if gi + 2 < n:
                    st[gi + 2] = emit_load((*groups[gi + 2], gi + 2))
                stage(gi, emit_exp)
                stage(gi + 1, emit_fadd)
                stage(gi + 1, emit_logits)
                stage(gi - 2, emit_epilogue)
                stage(gi, emit_qex_mm)
                stage(gi - 2, emit_epi_b)
                stage(gi + 2, emit_fexp_tx)
                if gi - 2 >= 0:
                    st[gi - 2] = None
            for gi in (n - 2, n - 1):
                stage(gi, emit_epilogue)
                stage(gi, emit_epi_b)

    nc.finalize()
    return nc


def _host_arrays(query, key, attn_kernel, targets):
    _ensure_imports()
    import concourse.mybir as mybir

    bf16 = mybir.dt.np(mybir.dt.bfloat16)
    perms, rels, node_order, n_win = preprocess(targets)
    n_slots = n_win * CAP

    # h-major feature columns on device: col c*8+h holds head-h channel-c.
    # COLPERM is an involution (8x8 transpose), so it also un-permutes.
    colperm = np.arange(HC).reshape(H, C).T.reshape(-1)
    wrow_1 = attn_kernel.reshape(-1)  # [c*8+h] = A[c,h]
    wrow = np.tile(wrow_1, (SUB, G * TW)).astype(np.float16)
    # W128x4 for the PE logits path: rows r = (qk half, c, h') with value
    # A[c,h] at col j==h' (cols 8..31 zero so all 32 PSUM rows are written)
    rr = np.arange(2 * HC)
    w128 = np.zeros((2 * HC, 32), dtype=np.float16)
    w128[rr, rr % H] = attn_kernel[(rr % HC) // H, rr % H].astype(np.float16)

    groups = _groups(n_win)
    splits = [_split(TW * g) for _, g in groups]
    # E-subtile ids (global, group order) and F-subtile column order:
    # per group, per chunk of PF subtiles, per band a, per subtile, the 32
    # edge slots [32a, 32a+32) -> one contiguous 2D matmul rhs per band.
    e_ids, f_ids, f_slot_cols = [], [], []
    for (w0, g), (nE, nF) in zip(groups, splits):
        s0 = w0 * TW
        e_ids.extend(range(s0, s0 + nE))
        f_ids.extend(range(s0 + nE, s0 + nE + nF))
        for ci in range(nF // PF):
            sids = s0 + nE + ci * PF + np.arange(PF)
            cols = (sids[None, :, None] * SUB
                    + 32 * np.arange(4)[:, None, None]
                    + np.arange(32)[None, None, :])
            f_slot_cols.append(cols.reshape(-1))
    e_ids = np.asarray(e_ids, dtype=np.int64)
    f_ids = np.asarray(f_ids, dtype=np.int64)
    f_slot_cols = (np.concatenate(f_slot_cols) if f_slot_cols
                   else np.zeros(0, dtype=np.int64))

    q16 = query[:, colperm].astype(np.float16)
    k16 = key[:, colperm].astype(np.float16)
    jj = np.arange(WIN, dtype=np.int8)
    in_maps = []
    for c in range(N_CORES):
        sel = perms[c]
        valid = sel >= 0
        qkc = np.zeros((n_slots, 2 * HC), dtype=np.float16)
        qkc[valid, :HC] = q16[sel[valid]]
        qkc[valid, HC:] = k16[sel[valid]]
        qkc3 = qkc.reshape(n_win * TW, SUB, 2 * HC)
        # E-subtiles, edge-major: [p, (e_ord c)]
        qke_til = np.ascontiguousarray(
            qkc3[e_ids].transpose(1, 0, 2).reshape(SUB, -1))
        # F-subtiles, feat-major: [feature row, (chunk a subtile edge)]
        if len(f_slot_cols):
            qkf_til = np.ascontiguousarray(qkc[f_slot_cols, :].T)
            qe_til = np.ascontiguousarray(
                qkc3[f_ids][:, :, 0:HC].transpose(1, 0, 2).reshape(SUB, -1))
        else:
            qkf_til = np.zeros((SUB, SUB), dtype=np.float16)
            qe_til = np.zeros((SUB, HC), dtype=np.float16)
        # one-hot selector, pre-tiled: [p, (w t j)], fp8e4 (0/1 exact)
        f8np = mybir.dt.np(mybir.dt.float8e4)
        onehot = (rels[c][:, None] == jj[None, :])  # [n_slots, WIN] bool
        s_til = np.ascontiguousarray(
            onehot.reshape(n_win * TW, SUB, WIN).transpose(1, 0, 2)
            .reshape(SUB, n_win * TW * WIN)).astype(np.float32).astype(f8np)
        in_maps.append({
            "qk": qke_til,
            "qkf": qkf_til,
            "qe": qe_til,
            "w128": w128,
            "sel": s_til,
            "wrow": wrow,
        })
    return in_maps, node_order, n_win


TRACE = False          # set by test harness to capture an NTFF profile
TRACE_CORES = None
LAST_RESULTS = None    # BassKernelResults of the most recent run


def kernel(query, key, attn_kernel, targets):
    global LAST_RESULTS
    query = np.asarray(query, dtype=np.float32)
    key = np.asarray(key, dtype=np.float32)
    attn_kernel = np.asarray(attn_kernel, dtype=np.float32)
    targets = np.asarray(targets, dtype=np.int32)

    _ensure_imports()
    from concourse.bass_utils import run_bass_kernel_spmd

    in_maps, node_order, n_win = _host_arrays(
        query, key, attn_kernel, targets)
    nc = build_nc(n_win)
    res = run_bass_kernel_spmd(
        nc, in_maps, list(range(N_CORES)),
        trace=TRACE, trace_cores=TRACE_CORES,
    )
    LAST_RESULTS = res
    colperm = np.arange(HC).reshape(H, C).T.reshape(-1)
    out = np.zeros((N_NODES, HC), dtype=np.float32)
    for c in range(N_CORES):
        # out dram [WIN, n_win*HC] -> rows (w*WIN + p); cols are h-major
        oc = np.asarray(res.results[c]["out"]).astype(np.float32)
        oc = oc.reshape(WIN, n_win, HC).transpose(1, 0, 2) \
            .reshape(n_win * WIN, HC)[:, colperm]
        rows = node_order[c]
        vmask = rows >= 0
        out[rows[vmask]] = oc[vmask]

    deg = np.bincount(targets, minlength=N_NODES)
    out[deg == 0] = 0.0
    return out



# revision 24
# speedup vs baseline: 1.1021x; 1.1021x over previous
"""GATv2 attention-pool kernel for 8 Trainium2 NeuronCores.

Algorithm
---------
Reference computes, per edge e with target node t(e):
    feats = q + k                                   [E, 64]
    logits[e,h] = sum_c feats[e,h*8+c] * A[c,h]     [E, 8]
    attn = segment_softmax(logits, targets)         [E, 8]
    out[n] = relu(segment_sum(q * attn))            [N, 64]

Logits are O(10) so exp() never overflows fp32/bf16; the segment-max shift
is unnecessary and softmax folds into two segment-SUMS sharing one pass:
    denom[n,h]  = sum_{e->n} exp(logits[e,h])
    pooled[n,:] = sum_{e->n} q[e,:] * exp(logits[e,h])
    out[n]      = relu(pooled[n]) / denom[n]        (relu commutes: denom>0)

Distribution: edges partitioned by target node (host-side sort), 100000
nodes split into 8 contiguous shards -> all segment reductions core-local,
no collectives.  Each shard's nodes are LPT-packed into windows of <= 32
nodes and <= 512 edges (4 subtiles of 128, ~0.6% slot padding); per
subtile the PE accumulates
    psum[32, 72] += S^T @ [q*ex | ex]
over the window's subtiles (S = host-built one-hot selector, streamed like
the data), then relu/divide once per node.

Key performance choices (vs a naive port):
- fp16 staging of q/k and bf16 ex/matmul operands halve HBM traffic and
  double DVE throughput (2x_1p mode needs 2-byte packed operands);
  rel-err vs the fp32 reference is ~8e-3 (tolerance 2e-2).
- fp8e4 one-hot selector: 0/1 is exact in fp8 and the PE accepts an fp8
  stationary operand against a bf16 moving one (HW-probed, err 2e-3);
  halves selector HBM traffic.
- HYBRID E/F split (the big one): the system is simultaneously DVE- and
  DMA-limited.  The last NF of each group's 56 subtiles ("F-subtiles")
  compute logits on the PE instead of DVE: a feat-major copy of q,k
  (features on partitions) is matmul'd against a zero-padded [128,32]
  weight whose band-a matmul lands at PSUM partitions [32a,32a+32)
  via tile_position, so after ACT exp (PSUM->SBUF) a DVE 32x32 block
  stream-transpose yields edge-major ex directly.  This removes
  fadd/wmul/tree (~94ns/subtile of DVE) at +16KB/subtile of DMA
  (feat-major qk + an edge-major q copy for qex).  NF balances the two
  rooflines.  The F chain runs TWO group-iterations ahead (matmuls at
  load time, exp+transpose emitted at iteration end) so the in-order
  DVE queue never waits on the PE->ACT->DVE chain.
- DMA split across BOTH HWDGE rings (SP + ACT) plus SWDGE (gpsimd) for
  the out-writes: a single ring head-of-line blocks loads behind the
  out-DMA's wait on the epilogue, capping sustained DMA at ~260GB/s;
  split rings reach ~290-350GB/s.
- h-major feature columns (c outer, h inner) make the per-head exp()
  broadcast and the 1/denom broadcast middle-axis: every DVE operand
  keeps a packed innermost dim and no broadcast is ever materialized.
- the c-reduction for E-subtile logits is a 3-step contiguous-halves
  add tree (tensor_reduce has no fast DVE mode; the tree runs at 2x).
- elementwise work runs on DVE; ACT does exp/relu, PE the matmuls.

Host work is index metadata + data layout only (argsort/packing of
targets, one-hot selector layout, gather + dtype cast of q/k rows into
slot order); all floating-point arithmetic runs on device.

Measured dead ends, do not retry blindly: GpSimd tensor-op offload
(qex share on Pool: TT runs 3.9ns/el AND concurrent Pool SBUF traffic
poisons DVE ops 2000->4200ns; 262->356us), per-subtile PE transposes
(LDWEIGHTS-bound), tensor_tensor divide (codegen reject), ACT Ln/Exp
reciprocal (crashes the core), fp8 staging of q/k (precision).
DVE per-op cost model that matches HW within 5%: (58 + FD/2)/0.96GHz
for 16-bit tensor_tensor (2x_1P cap), 1x for stream transpose/scan.
"""

import os
import sys

import numpy as np

N_NODES = 100000
N_EDGES = 1600000
H = 8
C = 8
HC = H * C
N_CORES = 8
SUB = 128
WIN = 32          # nodes per window
TW = 4            # subtiles per window
CAP = TW * SUB    # max edges per window
G = 14            # windows per device group (2 psum tiles)
PWIN = 7          # windows per psum tile ((7*72)*4B < 2KB bank)

FADD_DVE_SUBTILES = 999  # f-add subtiles on DVE (rest on GpSimd).  All-DVE
                         # measured fastest: any GpSimd share re-couples the
                         # Vector pipeline to the slow Q7 engine.
QEX_POOL_NUM = 0         # qex subtiles (of 56/group) computed on GpSimd.
                         # MEASURED DEAD END at 30: Pool TT runs 3.9ns/el
                         # (not 2.2) AND concurrent Pool traffic poisons DVE
                         # via the shared SBUF port (fadd/wmul 2000->4200ns;
                         # total 262->356us).  Keep 0.
DMUL_ENGINE = "vector"   # engine for out = relu(pooled) * rc ("gpsimd"
                         # measured slower, same port-contention mechanism)

# Hybrid E/F split: the LAST `NF` subtiles (per 56-subtile group) are
# "F-subtiles": their logits run on the PE from a feat-major copy of q,k
# (one matmul per 32-edge band, placed at PSUM partition base 32a via
# tile_position), exp on ACT straight from PSUM, and a DVE 32x32
# stream-transpose flips ex back to edge-major.  This removes fadd/wmul/
# tree from DVE for those subtiles at the cost of loading q twice for
# them (feat-major inside qkF + edge-major qE).  DVE is the bottleneck
# (245us busy of 262); the extra DMA rides unused DMA-engine headroom.
NF = 12                  # F-subtiles per full 56-subtile group
PF = 12                  # F-subtiles per PSUM chunk (12*32*4B = 1536B/bank)


def _split(Tg):
    nf = (Tg * NF // (G * TW)) // PF * PF
    return Tg - nf, nf


def _ensure_imports():
    try:
        import concourse.bass  # noqa: F401
    except ImportError:
        for p in ("/opt/trn_rl_repo", "/root/.axon_site/_ro/trn_rl_repo"):
            if os.path.isdir(p) and p not in sys.path:
                sys.path.insert(0, p)


def preprocess(targets):
    """Sort edges by target; LPT-pack each core's nodes into windows.

    Windows hold <= WIN nodes and <= CAP edges.  LPT (assign nodes in
    descending degree order to the least-loaded feasible window) packs to
    ~0.6% slot padding.  Returns (perms [n_cores, n_slots] edge ids (-1 =
    padding), rels [n_cores, n_slots] int8 local node id (-1 = padding),
    node_order [n_cores, n_win*WIN] node id per output row (-1 = unused),
    n_win).
    """
    import heapq

    npc = N_NODES // N_CORES
    order = np.argsort(targets, kind="stable")
    tsorted = targets[order]
    node_start = np.searchsorted(tsorted, np.arange(N_NODES + 1))
    deg = np.diff(node_start)

    def lpt(nodes, n_win):
        degs = deg[nodes]
        heap = [(0, 0, i) for i in range(n_win)]
        heapq.heapify(heap)
        assign = [[] for _ in range(n_win)]
        for nd in np.argsort(-degs, kind="stable"):
            dd = int(degs[nd])
            popped = []
            placed = False
            while heap:
                load, cnt, i = heapq.heappop(heap)
                if load + dd <= CAP and cnt + 1 <= WIN:
                    heapq.heappush(heap, (load + dd, cnt + 1, i))
                    assign[i].append(int(nodes[nd]))
                    placed = True
                    break
                popped.append((load, cnt, i))
            for p in popped:
                heapq.heappush(heap, p)
            if not placed:
                return None
        return assign

    # minimal feasible n_win per core, then re-pack all cores at the max
    packs, n_wins = [], []
    for c in range(N_CORES):
        nodes = np.arange(c * npc, (c + 1) * npc)
        n_win = int(np.ceil(max(deg[nodes].sum() / CAP, len(nodes) / WIN)))
        while True:
            a = lpt(nodes, n_win)
            if a is not None:
                break
            n_win += 1
        n_wins.append(n_win)
    n_win = max(n_wins)
    for c in range(N_CORES):
        nodes = np.arange(c * npc, (c + 1) * npc)
        a = lpt(nodes, n_win)
        assert a is not None
        packs.append(a)

    n_slots = n_win * CAP
    perms = np.full((N_CORES, n_slots), -1, dtype=np.int64)
    rels = np.full((N_CORES, n_slots), -1, dtype=np.int8)
    node_order = np.full((N_CORES, n_win * WIN), -1, dtype=np.int64)
    for c in range(N_CORES):
        for w, cur in enumerate(packs[c]):
            sb = w * CAP
            pos = 0
            for j, node in enumerate(cur):
                e0, e1 = node_start[node], node_start[node + 1]
                cnt = e1 - e0
                perms[c, sb + pos:sb + pos + cnt] = order[e0:e1]
                rels[c, sb + pos:sb + pos + cnt] = j
                pos += cnt
                node_order[c, w * WIN + j] = node
    return perms, rels, node_order, n_win


def _groups(n_win):
    gs, w0 = [], 0
    while w0 < n_win:
        g = min(G, n_win - w0)
        gs.append((w0, g))
        w0 += g
    return gs


def build_nc(n_win):
    """Build the single SPMD Bass program for one core's shard."""
    _ensure_imports()
    import concourse.bacc as bacc
    import concourse.mybir as mybir
    import concourse.tile as tile

    f32 = mybir.dt.float32
    f16 = mybir.dt.float16
    bf16 = mybir.dt.bfloat16
    f8 = mybir.dt.float8e4

    groups = _groups(n_win)
    AF = mybir.ActivationFunctionType
    OP = mybir.AluOpType

    splits = [_split(TW * g) for _, g in groups]
    offE = np.concatenate([[0], np.cumsum([e for e, _ in splits])])
    offF = np.concatenate([[0], np.cumsum([f for _, f in splits])])
    totE, totF = int(offE[-1]), int(offF[-1])

    nc = bacc.Bacc("TRN2", num_devices=N_CORES)
    qkD = nc.declare_dram_parameter("qk", [SUB, totE * SUB], f16, False)
    qkFD = nc.declare_dram_parameter(
        "qkf", [SUB, max(totF, 1) * SUB], f16, False)
    qED = nc.declare_dram_parameter(
        "qe", [SUB, max(totF, 1) * HC], f16, False)
    w128D = nc.declare_dram_parameter("w128", [SUB, 32], f16, False)
    # fp8e4 selector: one-hot 0/1 is exact in fp8 and the PE accepts an
    # fp8 stationary operand against a bf16 moving one (probed on HW,
    # rel err 2e-3) -> halves the selector's HBM traffic.
    sD = nc.declare_dram_parameter("sel", [SUB, n_win * TW * WIN], f8,
                                   False)
    wD = nc.declare_dram_parameter("wrow", [SUB, G * TW * HC], f16, False)
    outD = nc.declare_dram_parameter(
        "out", [WIN, n_win * HC], bf16, isOutput=True)

    with tile.TileContext(nc) as tc:
        with (
            tc.tile_pool(name="const", bufs=1) as cpool,
            tc.tile_pool(name="qk", bufs=4) as qkpool,
            tc.tile_pool(name="mid", bufs=3) as midpool,
            tc.tile_pool(name="mm", bufs=3) as mmpool,
            tc.tile_pool(name="fin", bufs=3) as finpool,
            tc.tile_pool(name="psum", bufs=6, space="PSUM") as ppool,
            tc.tile_pool(name="psumL", bufs=2, space="PSUM") as plpool,
        ):
            w_t = cpool.tile([SUB, G * TW * HC], f16)
            nc.sync.dma_start(out=w_t[:], in_=wD[:])
            w128_t = cpool.tile([SUB, 32], f16)
            nc.sync.dma_start(out=w128_t[:], in_=w128D[:])

            def emit_load(grp):
                w0, g, gk = grp
                Tg = TW * g
                nE, nF = splits[gk]
                st = {"grp": grp, "Tg": Tg, "nE": nE, "nF": nF}
                # qkF/qE first: the F logits chain consumes qkF this same
                # iteration (2 periods before the group's DVE stage)
                # two HWDGE rings in parallel: qkF+sel on the ACT ring,
                # qkE+qE on the SP ring, out-writes on SWDGE (gpsimd) --
                # a single ring head-of-line blocks loads behind the
                # out-DMA's wait on the epilogue.  qkE first: the E chain
                # (fadd) is the rampup critical path.
                if nE > 0:
                    qk_t = qkpool.tile([SUB, nE * SUB], f16, tag=f"qk{nE}")
                    nc.sync.dma_start(
                        out=qk_t[:],
                        in_=qkD[:, offE[gk] * SUB:(offE[gk] + nE) * SUB])
                    st["qk3"] = qk_t[:].rearrange("p (t c) -> p t c", c=SUB)
                if nF > 0:
                    qkF_t = qkpool.tile([SUB, nF * SUB], f16, tag=f"qkF{nF}")
                    nc.scalar.dma_start(
                        out=qkF_t[:],
                        in_=qkFD[:, offF[gk] * SUB:(offF[gk] + nF) * SUB])
                    qE_t = qkpool.tile([SUB, nF * HC], f16, tag=f"qE{nF}")
                    nc.sync.dma_start(
                        out=qE_t[:],
                        in_=qED[:, offF[gk] * HC:(offF[gk] + nF) * HC])
                    st["qkF"] = qkF_t
                    st["qE3"] = qE_t[:].rearrange("p (t c) -> p t c", c=HC)
                s_t = qkpool.tile([SUB, Tg, WIN], f8, tag=f"S{g}")
                nc.scalar.dma_start(
                    out=s_t[:], in_=sD[:, w0 * TW * WIN:(w0 + g) * TW * WIN])
                st["S"] = s_t
                # F-subtile logits on PE, immediately at load time: per
                # chunk of PF subtiles, band a covers edges [32a, 32a+32) of
                # each subtile and lands at PSUM partitions [32a, 32a+32)
                # (tile_position) so a 32x32 stream-transpose yields
                # edge-major ex later.
                pls = []
                for ci in range(st["nF"] // PF):
                    pl = plpool.tile([SUB, PF * 32], f32, tag="PL")
                    for a in range(4):
                        c0 = (ci * 4 + a) * PF * 32
                        nc.tensor.matmul(
                            pl[a * 32:(a + 1) * 32, :],
                            lhsT=w128_t[:],
                            rhs=qkF_t[:, c0:c0 + PF * 32],
                            start=True, stop=True,
                            tile_position=(0, a * 32),
                        )
                    pls.append(pl)
                st["pls"] = pls
                return st

            def emit_fexp_tx(s):
                # tail of the F logits chain (still 2 periods ahead of the
                # group's DVE stage): exp from PSUM on ACT, then the 32x32
                # block stream-transpose to edge-major.  Emitted at the very
                # END of the iteration so nothing this period waits on it.
                txs = []
                for pl in s["pls"]:
                    px = mmpool.tile([SUB, PF * 32], bf16, tag="PX")
                    nc.scalar.activation(out=px[:], in_=pl[:], func=AF.Exp)
                    tx = mmpool.tile([SUB, PF * 32], bf16, tag="TX", bufs=6)
                    nc.vector.transpose(tx[:], px[:])
                    txs.append(tx)
                s["txs"] = txs
                s["pls"] = None

            def emit_fadd(s):
                nE = s["nE"]
                if nE == 0:
                    return
                f_t = midpool.tile([SUB, nE * HC], f16, tag=f"f{nE}", bufs=3)
                fv = f_t[:].rearrange("p (t c) -> p t c", c=HC)
                nc.vector.tensor_add(
                    fv[:], s["qk3"][:, :, 0:HC], s["qk3"][:, :, HC:2 * HC])
                s["f"] = f_t

            def emit_logits(s):
                # E-subtiles: features are h-major (c outer, h inner): the
                # c-reduction tree adds contiguous halves (2x DVE mode)
                nE, nF = s["nE"], s["nF"]
                if nE > 0:
                    wf_t = midpool.tile([SUB, nE * HC], f16, tag=f"wf{nE}")
                    nc.vector.tensor_mul(wf_t[:], s["f"][:], w_t[:, :nE * HC])
                    wfv = wf_t[:].rearrange("p (t c) -> p t c", c=HC)
                    t1 = midpool.tile([SUB, nE, 32], f16, tag=f"t1{nE}")
                    nc.vector.tensor_add(
                        t1[:], wfv[:, :, 0:32], wfv[:, :, 32:64])
                    t2 = midpool.tile([SUB, nE, 16], f16, tag=f"t2{nE}")
                    nc.vector.tensor_add(
                        t2[:], t1[:, :, 0:16], t1[:, :, 16:32])
                    lg = midpool.tile([SUB, nE * H], f16, tag=f"lg{nE}")
                    nc.vector.tensor_add(
                        lg[:].rearrange("p (t h) -> p t h", h=H),
                        t2[:, :, 0:8], t2[:, :, 8:16])
                    s["lg"] = lg

            def emit_exp(s):
                Tg, nE = s["Tg"], s["nE"]
                m_t = mmpool.tile([SUB, Tg, HC + H], bf16, tag=f"M{Tg}")
                if nE > 0:
                    lg3 = s["lg"][:].rearrange("p (t h) -> p t h", h=H)
                    nc.scalar.activation(
                        out=m_t[:, 0:nE, HC:HC + H], in_=lg3, func=AF.Exp)
                for ci, tx in enumerate(s["txs"]):
                    t0 = nE + ci * PF
                    nc.vector.tensor_copy(
                        m_t[:, t0:t0 + PF, HC:HC + H],
                        tx[:].rearrange("p (t c) -> p t c", c=32)[:, :, 0:H])
                s["m"] = m_t

            def emit_qex_mm(s):
                # h-major: the per-head ex broadcast is along the MIDDLE (c)
                # axis, so every operand's innermost dim stays packed (2x)
                # and no materialized broadcast is needed.
                Tg, nE, nF = s["Tg"], s["nE"], s["nF"]
                m_t = s["m"]
                exv = m_t[:, :, HC:HC + H]
                if nE > 0:
                    nc.vector.tensor_mul(
                        m_t[:, 0:nE, 0:HC].rearrange(
                            "p t (cc h) -> p t cc h", h=H),
                        s["qk3"][:, :, 0:HC].rearrange(
                            "p t (cc h) -> p t cc h", h=H),
                        exv[:, 0:nE, None, :].to_broadcast([SUB, nE, C, H]),
                    )
                if nF > 0:
                    nc.vector.tensor_mul(
                        m_t[:, nE:Tg, 0:HC].rearrange(
                            "p t (cc h) -> p t cc h", h=H),
                        s["qE3"][:].rearrange(
                            "p t (cc h) -> p t cc h", h=H),
                        exv[:, nE:Tg, None, :].to_broadcast([SUB, nF, C, H]),
                    )
                w0, g, _ = s["grp"]
                n_ps = (g + PWIN - 1) // PWIN
                ps = []
                for pi in range(n_ps):
                    nw = min(PWIN, g - pi * PWIN)
                    p_t = ppool.tile([WIN, PWIN * (HC + H)], f32, tag="ps")
                    for wi in range(nw):
                        sub0 = (pi * PWIN + wi) * TW
                        pcols = slice(wi * (HC + H), (wi + 1) * (HC + H))
                        for t in range(TW):
                            nc.tensor.matmul(
                                p_t[:, pcols],
                                lhsT=s["S"][:, sub0 + t, :],
                                rhs=m_t[:, sub0 + t, :],
                                start=(t == 0),
                                stop=(t == TW - 1),
                            )
                    ps.append((p_t, nw))
                s["ps"] = ps

            def emit_epilogue(s):
                w0, g, _ = s["grp"]
                po = finpool.tile([WIN, g, HC + H], bf16, tag=f"po{g}")
                off = 0
                for p_t, nw in s["ps"]:
                    nc.scalar.activation(
                        out=po[:, off:off + nw, :],
                        in_=p_t[:, :nw * (HC + H)].rearrange(
                            "p (w j) -> p w j", j=HC + H),
                        func=AF.Relu,
                    )
                    off += nw
                # 1/denom via the single-pass approx (~18 bits, plenty for
                # the bf16 result); it needs fp32 in/out, staged through the
                # idle ACT engine.  denom>0 for every real node (no
                # zero-degree nodes reach here; padded rows are host-masked).
                dn = finpool.tile([WIN, g, H], f32, tag=f"dn{g}")
                nc.scalar.activation(out=dn[:], in_=po[:, :, HC:HC + H],
                                     func=AF.Copy)
                rcf = finpool.tile([WIN, g, H], f32, tag=f"rcf{g}")
                nc.vector.reciprocal_approx_fast(out=rcf[:], in_=dn[:])
                rc = finpool.tile([WIN, g, H], bf16, tag=f"rc{g}")
                nc.scalar.activation(out=rc[:], in_=rcf[:], func=AF.Copy)
                s["po"], s["rc"] = po, rc

            def emit_epi_b(s):
                w0, g, _ = s["grp"]
                o_t = finpool.tile([WIN, g, HC], bf16, tag=f"o{g}")
                dmul_eng = nc.gpsimd if DMUL_ENGINE == "gpsimd" else nc.vector
                dmul_eng.tensor_mul(
                    o_t[:].rearrange("p w (cc h) -> p w cc h", h=H),
                    s["po"][:, :, 0:HC].rearrange(
                        "p w (cc h) -> p w cc h", h=H),
                    s["rc"][:, :, None, :].to_broadcast([WIN, g, C, H]),
                )
                nc.gpsimd.dma_start(
                    out=outD[:, w0 * HC:(w0 + g) * HC], in_=o_t[:])

            # 3-deep software pipeline; epilogue runs 2 iterations behind so
            # no engine's first op of an iteration waits on a fresh product.
            # Steady-state per-iteration engine FIFOs:
            #   ACT:  relu(i-2), exp/expb(i)
            #   DVE:  recip(i-2), wmul/tree(i+1), fadd-share(i+2), qex(i)
            #   Pool: dmul(i-2), fadd-share(i+2)
            #   PE:   mm(i);  DMA: out(i-2), load(i+3)
            n = len(groups)
            st = [None] * n

            def stage(gi, fn):
                if 0 <= gi < n:
                    fn(st[gi])

            for gi in range(min(2, n)):
                st[gi] = emit_load((*groups[gi], gi))
            stage(0, emit_fexp_tx)
            stage(1, emit_fexp_tx)
            stage(0, emit_fadd)
            stage(0, emit_logits)
            for gi in range(n):
                if gi + 2 < n:
                    st[gi + 2] = emit_load((*groups[gi + 2], gi + 2))
                stage(gi, emit_exp)
                stage(gi + 1, emit_fadd)
                stage(gi + 1, emit_logits)
                stage(gi - 2, emit_epilogue)
                stage(gi, emit_qex_mm)
                stage(gi - 2, emit_epi_b)
                stage(gi + 2, emit_fexp_tx)
                if gi - 2 >= 0:
                    st[gi - 2] = None
            for gi in (n - 2, n - 1):
                stage(gi, emit_epilogue)
                stage(gi, emit_epi_b)

    nc.finalize()
    return nc


def _host_arrays(query, key, attn_kernel, targets):
    _ensure_imports()
    import concourse.mybir as mybir

    bf16 = mybir.dt.np(mybir.dt.bfloat16)
    perms, rels, node_order, n_win = preprocess(targets)
    n_slots = n_win * CAP

    # h-major feature columns on device: col c*8+h holds head-h channel-c.
    # COLPERM is an involution (8x8 transpose), so it also un-permutes.
    colperm = np.arange(HC).reshape(H, C).T.reshape(-1)
    wrow_1 = attn_kernel.reshape(-1)  # [c*8+h] = A[c,h]
    wrow = np.tile(wrow_1, (SUB, G * TW)).astype(np.float16)
    # W128x4 for the PE logits path: rows r = (qk half, c, h') with value
    # A[c,h] at col j==h' (cols 8..31 zero so all 32 PSUM rows are written)
    rr = np.arange(2 * HC)
    w128 = np.zeros((2 * HC, 32), dtype=np.float16)
    w128[rr, rr % H] = attn_kernel[(rr % HC) // H, rr % H].astype(np.float16)

    groups = _groups(n_win)
    splits = [_split(TW * g) for _, g in groups]
    # E-subtile ids (global, group order) and F-subtile column order:
    # per group, per chunk of PF subtiles, per band a, per subtile, the 32
    # edge slots [32a, 32a+32) -> one contiguous 2D matmul rhs per band.
    e_ids, f_ids, f_slot_cols = [], [], []
    for (w0, g), (nE, nF) in zip(groups, splits):
        s0 = w0 * TW
        e_ids.extend(range(s0, s0 + nE))
        f_ids.extend(range(s0 + nE, s0 + nE + nF))
        for ci in range(nF // PF):
            sids = s0 + nE + ci * PF + np.arange(PF)
            cols = (sids[None, :, None] * SUB
                    + 32 * np.arange(4)[:, None, None]
                    + np.arange(32)[None, None, :])
            f_slot_cols.append(cols.reshape(-1))
    e_ids = np.asarray(e_ids, dtype=np.int64)
    f_ids = np.asarray(f_ids, dtype=np.int64)
    f_slot_cols = (np.concatenate(f_slot_cols) if f_slot_cols
                   else np.zeros(0, dtype=np.int64))

    q16 = query[:, colperm].astype(np.float16)
    k16 = key[:, colperm].astype(np.float16)
    jj = np.arange(WIN, dtype=np.int8)
    in_maps = []
    for c in range(N_CORES):
        sel = perms[c]
        valid = sel >= 0
        qkc = np.zeros((n_slots, 2 * HC), dtype=np.float16)
        qkc[valid, :HC] = q16[sel[valid]]
        qkc[valid, HC:] = k16[sel[valid]]
        qkc3 = qkc.reshape(n_win * TW, SUB, 2 * HC)
        # E-subtiles, edge-major: [p, (e_ord c)]
        qke_til = np.ascontiguousarray(
            qkc3[e_ids].transpose(1, 0, 2).reshape(SUB, -1))
        # F-subtiles, feat-major: [feature row, (chunk a subtile edge)]
        if len(f_slot_cols):
            qkf_til = np.ascontiguousarray(qkc[f_slot_cols, :].T)
            qe_til = np.ascontiguousarray(
                qkc3[f_ids][:, :, 0:HC].transpose(1, 0, 2).reshape(SUB, -1))
        else:
            qkf_til = np.zeros((SUB, SUB), dtype=np.float16)
            qe_til = np.zeros((SUB, HC), dtype=np.float16)
        # one-hot selector, pre-tiled: [p, (w t j)], fp8e4 (0/1 exact)
        f8np = mybir.dt.np(mybir.dt.float8e4)
        onehot = (rels[c][:, None] == jj[None, :])  # [n_slots, WIN] bool
        s_til = np.ascontiguousarray(
            onehot.reshape(n_win * TW, SUB, WIN).transpose(1, 0, 2)
            .reshape(SUB, n_win * TW * WIN)).astype(np.float32).astype(f8np)
        in_maps.append({
            "qk": qke_til,
            "qkf": qkf_til,
            "qe": qe_til,
            "w128": w128,
            "sel": s_til,
            "wrow": wrow,
        })
    return in_maps, node_order, n_win


TRACE = False          # set by test harness to capture an NTFF profile
TRACE_CORES = None
LAST_RESULTS = None    # BassKernelResults of the most recent run


def kernel(query, key, attn_kernel, targets):
    global LAST_RESULTS
    query = np.asarray(query, dtype=np.float32)
    key = np.asarray(key, dtype=np.float32)
    attn_kernel = np.asarray(attn_kernel, dtype=np.float32)
    targets = np.asarray(targets, dtype=np.int32)

    _ensure_imports()
    from concourse.bass_utils import run_bass_kernel_spmd

    in_maps, node_order, n_win = _host_arrays(
        query, key, attn_kernel, targets)
    nc = build_nc(n_win)
    res = run_bass_kernel_spmd(
        nc, in_maps, list(range(N_CORES)),
        trace=TRACE, trace_cores=TRACE_CORES,
    )
    LAST_RESULTS = res
    colperm = np.arange(HC).reshape(H, C).T.reshape(-1)
    out = np.zeros((N_NODES, HC), dtype=np.float32)
    for c in range(N_CORES):
        # out dram [WIN, n_win*HC] -> rows (w*WIN + p); cols are h-major
        oc = np.asarray(res.results[c]["out"]).astype(np.float32)
        oc = oc.reshape(WIN, n_win, HC).transpose(1, 0, 2) \
            .reshape(n_win * WIN, HC)[:, colperm]
        rows = node_order[c]
        vmask = rows >= 0
        out[rows[vmask]] = oc[vmask]

    deg = np.bincount(targets, minlength=N_NODES)
    out[deg == 0] = 0.0
    return out

